# revision 17
# baseline (speedup 1.0000x reference)
"""BlockSparseLinear on 8 TRN2 NeuronCores: out = x @ W^T + bias.

Harness entry point: kernel(**inputs) takes the FULL inputs
(x (8192,4096) f32, weight (4096,4096) f32, bias (4096,) f32) and
returns the FULL output (8192,4096) f32.

Strategy: 8-way data parallel over batch + one level of Strassen, all
in bf16. Per core the shard out^T[:, b] = W @ x^T[:, b] + bias is a
[1024 x 4096] @ [4096 x 4096] product; Strassen splits batch (a/b
halves of 512), contraction K (halves of 2048) and outputs O (halves
of 2048) in 2x2 blocks and runs 7 block-products instead of 8 -> PE
streaming work drops 12.5% below the dense bf16 floor.

  A11 = x_a[:,K1]  A12 = x_a[:,K2]  A21 = x_b[:,K1]  A22 = x_b[:,K2]
  M1=(A11+A22)(B11+B22) M2=(A21+A22)B11 M3=A11(B12-B22)
  M4=A22(B21-B11)       M5=(A11+A12)B22 M6=(A21-A11)(B11+B12)
  M7=(A12-A22)(B21+B22)
  C11=M1+M4-M5+M7  C12=M3+M5  C21=M2+M4  C22=M1-M2+M3+M6

The 7 weight-sum matrices T_i are precomputed on the host in fp32 and
rounded once to bf16 (56 MB streamed per core). The 5 x-sums are built
on the DVE from the x^T shard as it lands. Each of the 7 M-products
accumulates 16 k-tiles into its own full PSUM bank; after a chain
completes, the DVE folds that M into the SBUF accumulators c11..c22
(bias folded into the first read, one PSUM operand per op - PSUM has a
single DVE read port), freeing the bank for the next output block
while the PE runs the remaining chains. Measured numerically on the
real data the scheme lands at ~5e-3 rel err vs the 2e-2 gate.

Layouts (host pre-blocked so every DMA is contiguous per partition):
  xk   (4, 16, 128, 512)        x^T blocks [A11,A22,A21,A12] as
                                [blk, k_tile, k_local, b_half]   bf16
  w    (7, 16, 128, 16, 128)    T_i blocked [i, j_tile, k_local,
                                k_tile, o_local]                 bf16
  bias (128, 32)                [o_local, j_tile]                f32
  out  (32, 128, 1024)          out^T blocked [j_tile, o_local, b] f32
"""

import ml_dtypes
import numpy as np

import concourse.mybir as mybir
import concourse.tile as tile
from concourse import bacc
from concourse.bass_utils import run_bass_kernel_spmd

NCORES = 8
BATCH, INF, OUTF = 8192, 4096, 4096
B = BATCH // NCORES          # per-core batch (1024)
BH = B // 2                  # Strassen batch half (512)
KH = 16                      # k-tiles per K-half (of 128 features)
JH = 16                      # o-tiles per O-half (of 128 outputs)
JT = OUTF // 128             # 32 o-tiles total

F32 = mybir.dt.float32
BF16 = mybir.dt.bfloat16
ADD = mybir.AluOpType.add
SUB = mybir.AluOpType.subtract

_NC_CACHE = {}

# chain order i -> (moving operand, which accumulators consume M_i and
# with what sign). Moving operands: S1,S2,A11,A22,S5,S6,S7.
# c11 = M1+M4-M5+M7   c12 = M3+M5   c21 = M2+M4   c22 = M1-M2+M3+M6
CHAINS = [
    # (w index i, moving key, [(dst, sign), ...])
    (0, "s1", [("c11", +1), ("c22", +1)]),   # M1
    (2, "a11", [("c12", +1), ("c22", +1)]),  # M3
    (3, "a22", [("c11", +1), ("c21", +1)]),  # M4
    (1, "s2", [("c21", +1), ("c22", -1)]),   # M2
    (5, "s6", [("c22", +1)]),                # M6
    (4, "s5", [("c12", +1), ("c11", -1)]),   # M5
    (6, "s7", [("c11", +1)]),                # M7
]
# which bias column (j for O1, 16+j for O2) and out rows per dst
DSTS = {
    "c11": (0, 0),    # (o-half, batch-half)
    "c12": (1, 0),
    "c21": (0, 1),
    "c22": (1, 1),
}
# number of M-terms per dst (to know when it is complete)
NTERMS = {"c11": 4, "c12": 2, "c21": 2, "c22": 4}


def _build_nc():
    if "nc" in _NC_CACHE:
        return _NC_CACHE["nc"]
    nc = bacc.Bacc("TRN2", target_bir_lowering=False, debug=False,
                   num_devices=NCORES)
    x_d = nc.dram_tensor("xk", [4, KH, 128, BH], BF16, kind="ExternalInput")
    w_d = nc.dram_tensor("w", [7, JH, 128, KH, 128], BF16,
                         kind="ExternalInput")
    b_d = nc.dram_tensor("bias", [128, JT], F32, kind="ExternalInput")
    o_d = nc.dram_tensor("out", [JT, 128, B], BF16,
                         kind="ExternalOutput")

    with tile.TileContext(nc) as tc:
        with (
            tc.tile_pool(name="xpool", bufs=1) as xpool,
            tc.tile_pool(name="xtr", bufs=4) as xtr,
            tc.tile_pool(name="spool", bufs=1) as spool,
            tc.tile_pool(name="wpool", bufs=1) as wpool,
            tc.tile_pool(name="bpool", bufs=1) as bpool,
            tc.tile_pool(name="opool", bufs=2) as opool,
            tc.tile_pool(name="pspool", bufs=1, space="PSUM") as pspool,
        ):
            bias_t = bpool.tile([128, JT], F32, tag="bias", name="bias_t")
            nc.scalar.dma_start(out=bias_t[:], in_=b_d[:])

            # x^T blocks: A11, A22 stay resident; A21, A12 pass through
            # rotating tiles feeding the DVE sum builders.
            a11, a22 = [], []
            s1, s2, s5, s6, s7 = [], [], [], [], []
            for k in range(KH):
                t11 = xpool.tile([128, BH], BF16, tag=f"a11_{k}",
                                 name=f"a11_{k}")
                t22 = xpool.tile([128, BH], BF16, tag=f"a22_{k}",
                                 name=f"a22_{k}")
                nc.sync.dma_start(out=t11[:], in_=x_d[0, k])
                nc.gpsimd.dma_start(out=t22[:], in_=x_d[1, k])
                a11.append(t11)
                a22.append(t22)
                st = spool.tile([128, BH], BF16, tag=f"s1_{k}",
                                name=f"s1_{k}")
                nc.vector.tensor_tensor(st[:], t11[:], t22[:], ADD)
                s1.append(st)
            for k in range(KH):
                t21 = xtr.tile([128, BH], BF16, tag="a21", name=f"a21_{k}")
                nc.gpsimd.dma_start(out=t21[:], in_=x_d[2, k])
                st2 = spool.tile([128, BH], BF16, tag=f"s2_{k}",
                                 name=f"s2_{k}")
                st6 = spool.tile([128, BH], BF16, tag=f"s6_{k}",
                                 name=f"s6_{k}")
                nc.vector.tensor_tensor(st2[:], t21[:], a22[k][:], ADD)
                nc.vector.tensor_tensor(st6[:], t21[:], a11[k][:], SUB)
                s2.append(st2)
                s6.append(st6)
            for k in range(KH):
                t12 = xtr.tile([128, BH], BF16, tag="a12", name=f"a12_{k}")
                nc.sync.dma_start(out=t12[:], in_=x_d[3, k])
                st5 = spool.tile([128, BH], BF16, tag=f"s5_{k}",
                                 name=f"s5_{k}")
                st7 = spool.tile([128, BH], BF16, tag=f"s7_{k}",
                                 name=f"s7_{k}")
                nc.vector.tensor_tensor(st5[:], a11[k][:], t12[:], ADD)
                nc.vector.tensor_tensor(st7[:], t12[:], a22[k][:], SUB)
                s5.append(st5)
                s7.append(st7)

            moving = {"s1": s1, "s2": s2, "a11": a11, "a22": a22,
                      "s5": s5, "s6": s6, "s7": s7}

            # chains run in pairs: consecutive matmuls alternate between
            # two chains (separate full PSUM banks), so each LDWEIGHTS
            # hides under the other chain's 512-col matmul.
            PAIRS = [(0, 1), (2, 3), (4, 5), (6,)]
            for j in range(JH):
                # stream the 7 stationary slabs for this o-tile, each in
                # 4 chunks so the first chain gates on 128 KB not 512 KB
                wts = {}
                for i, mv, _ in CHAINS:
                    wt = wpool.tile([128, KH, 128], BF16, tag=f"w{i}",
                                    name=f"w{i}")
                    for q in range(4):
                        nc.scalar.dma_start(
                            out=wt[:, 4 * q:4 * (q + 1), :],
                            in_=w_d[i, j, :, 4 * q:4 * (q + 1)])
                    wts[i] = wt

                acc = {}
                done = {d: 0 for d in NTERMS}
                for pair in PAIRS:
                    members = [CHAINS[c] for c in pair]
                    pss = {}
                    for i, mv, _ in members:
                        pss[i] = pspool.tile([128, BH], F32, tag=f"m{i}",
                                             name=f"m{i}")
                    for k in range(KH):
                        for i, mv, _ in members:
                            nc.tensor.matmul(
                                pss[i][:], wts[i][:, k, :],
                                moving[mv][k][:],
                                start=(k == 0), stop=(k == KH - 1),
                            )
                    # fold each M_i into its SBUF accumulators (frees
                    # the PSUM bank once the reads retire); the final
                    # fold of each accumulator writes bf16 to halve the
                    # output DMA traffic
                    for i, mv, dsts in members:
                        ps = pss[i]
                        for dst, sign in dsts:
                            ohalf, bhalf = DSTS[dst]
                            final = done[dst] + 1 == NTERMS[dst]
                            if dst not in acc:
                                acc[dst] = opool.tile([128, BH], F32,
                                                      tag=dst, name=dst)
                                assert sign > 0 and not final
                                nc.vector.tensor_scalar_add(
                                    acc[dst][:], ps[:],
                                    bias_t[:, 16 * ohalf + j:
                                           16 * ohalf + j + 1],
                                )
                            elif final:
                                ob = opool.tile([128, BH], BF16,
                                                tag=dst + "b",
                                                name=dst + "b")
                                nc.vector.tensor_tensor(
                                    ob[:], acc[dst][:], ps[:],
                                    ADD if sign > 0 else SUB,
                                )
                                nc.sync.dma_start(
                                    out=o_d[16 * ohalf + j, :,
                                            BH * bhalf:BH * (bhalf + 1)],
                                    in_=ob[:],
                                )
                            else:
                                nc.vector.tensor_tensor(
                                    acc[dst][:], acc[dst][:], ps[:],
                                    ADD if sign > 0 else SUB,
                                )
                            done[dst] += 1

    nc.compile()
    _NC_CACHE["nc"] = nc
    return nc


def kernel(x, weight, bias):
    x = np.asarray(x, dtype=np.float32)
    weight = np.asarray(weight, dtype=np.float32)
    bias = np.asarray(bias, dtype=np.float32)

    nc = _build_nc()

    # host-side Strassen weight sums (fp32 exact, single bf16 rounding)
    WT = weight.T  # [K, O]
    K1, K2 = slice(0, 2048), slice(2048, 4096)
    O1, O2 = slice(0, 2048), slice(2048, 4096)
    B11, B12 = WT[K1, O1], WT[K1, O2]
    B21, B22 = WT[K2, O1], WT[K2, O2]
    Ts = [B11 + B22, B11, B12 - B22, B21 - B11, B22, B11 + B12, B21 + B22]
    # [7, 2048 K, 2048 O] -> [7, j, k_local, k_tile, o_local]
    wr = np.empty((7, JH, 128, KH, 128), dtype=ml_dtypes.bfloat16)
    for i, T in enumerate(Ts):
        wr[i] = (T.astype(ml_dtypes.bfloat16)
                 .reshape(KH, 128, JH, 128).transpose(2, 1, 0, 3))
    br = np.ascontiguousarray(bias.reshape(JT, 128).T)

    in_maps = []
    for c in range(NCORES):
        xs = x[c * B:(c + 1) * B].astype(ml_dtypes.bfloat16)
        xb = np.empty((4, KH, 128, BH), dtype=ml_dtypes.bfloat16)
        for bi, (rs, cs) in enumerate(
                [(slice(0, BH), K1), (slice(BH, B), K2),
                 (slice(BH, B), K1), (slice(0, BH), K2)]):
            # order: A11, A22, A21, A12
            xb[bi] = xs[rs, cs].T.reshape(KH, 128, BH)
        in_maps.append({"xk": xb, "w": wr, "bias": br})

    res = run_bass_kernel_spmd(nc, in_maps, list(range(NCORES)))

    out = np.empty((BATCH, OUTF), np.float32)
    for c in range(NCORES):
        out[c * B:(c + 1) * B] = (res.results[c]["out"]
                                  .astype(np.float32).reshape(OUTF, B).T)
    return out


# revision 18
# speedup vs baseline: 1.1712x; 1.1712x over previous
"""BlockSparseLinear on 8 TRN2 NeuronCores: out = x @ W^T + bias.

Harness entry point: kernel(**inputs) takes the FULL inputs
(x (8192,4096) f32, weight (4096,4096) f32, bias (4096,) f32) and
returns the FULL output (8192,4096) f32.

Strategy: 8-way data parallel over batch + one level of Strassen, all
in bf16. Per core the shard out^T[:, b] = W @ x^T[:, b] + bias is a
[1024 x 4096] @ [4096 x 4096] product; Strassen splits batch (a/b
halves of 512), contraction K (halves of 2048) and outputs O (halves
of 2048) in 2x2 blocks and runs 7 block-products instead of 8 -> PE
streaming work drops 12.5% below the dense bf16 floor.

  A11 = x_a[:,K1]  A12 = x_a[:,K2]  A21 = x_b[:,K1]  A22 = x_b[:,K2]
  M1=(A11+A22)(B11+B22) M2=(A21+A22)B11 M3=A11(B12-B22)
  M4=A22(B21-B11)       M5=(A11+A12)B22 M6=(A21-A11)(B11+B12)
  M7=(A12-A22)(B21+B22)
  C11=M1+M4-M5+M7  C12=M3+M5  C21=M2+M4  C22=M1-M2+M3+M6

The 7 weight-sum matrices T_i are precomputed on the host in fp32 and
rounded once to bf16 (56 MB streamed per core). The 5 x-sums are built
on the DVE from the x^T shard as it lands. Each of the 7 M-products
accumulates 16 k-tiles into its own full PSUM bank; after a chain
completes, the DVE folds that M into the SBUF accumulators c11..c22
(bias folded into the first read, one PSUM operand per op - PSUM has a
single DVE read port), freeing the bank for the next output block
while the PE runs the remaining chains. Measured numerically on the
real data the scheme lands at ~5e-3 rel err vs the 2e-2 gate.

Layouts (host pre-blocked so every DMA is contiguous per partition):
  xk   (4, 16, 128, 512)        x^T blocks [A11,A22,A21,A12] as
                                [blk, k_tile, k_local, b_half]   bf16
  w    (7, 16, 128, 16, 128)    T_i blocked [i, j_tile, k_local,
                                k_tile, o_local]                 bf16
  bias (128, 32)                [o_local, j_tile]                f32
  out  (32, 128, 1024)          out^T blocked [j_tile, o_local, b] bf16
"""

import ml_dtypes
import numpy as np

import concourse.mybir as mybir
import concourse.tile as tile
from concourse import bacc
from concourse.bass_utils import run_bass_kernel_spmd

NCORES = 8
BATCH, INF, OUTF = 8192, 4096, 4096
B = BATCH // NCORES          # per-core batch (1024)
BH = B // 2                  # Strassen batch half (512)
KH = 16                      # k-tiles per K-half (of 128 features)
JH = 16                      # o-tiles per O-half (of 128 outputs)
JT = OUTF // 128             # 32 o-tiles total

F32 = mybir.dt.float32
BF16 = mybir.dt.bfloat16
ADD = mybir.AluOpType.add
SUB = mybir.AluOpType.subtract

_NC_CACHE = {}

# chain order i -> (moving operand, which accumulators consume M_i and
# with what sign). Moving operands: S1,S2,A11,A22,S5,S6,S7.
# c11 = M1+M4-M5+M7   c12 = M3+M5   c21 = M2+M4   c22 = M1-M2+M3+M6
CHAINS = [
    # (w index i, moving key, [(dst, sign), ...])
    (0, "s1", [("c11", +1), ("c22", +1)]),   # M1
    (2, "a11", [("c12", +1), ("c22", +1)]),  # M3
    (3, "a22", [("c11", +1), ("c21", +1)]),  # M4
    (1, "s2", [("c21", +1), ("c22", -1)]),   # M2
    (5, "s6", [("c22", +1)]),                # M6
    (4, "s5", [("c12", +1), ("c11", -1)]),   # M5
    (6, "s7", [("c11", +1)]),                # M7
]
# which bias column (j for O1, 16+j for O2) and out rows per dst
DSTS = {
    "c11": (0, 0),    # (o-half, batch-half)
    "c12": (1, 0),
    "c21": (0, 1),
    "c22": (1, 1),
}
# number of M-terms per dst (to know when it is complete)
NTERMS = {"c11": 4, "c12": 2, "c21": 2, "c22": 4}


def _build_nc():
    if "nc" in _NC_CACHE:
        return _NC_CACHE["nc"]
    nc = bacc.Bacc("TRN2", target_bir_lowering=False, debug=False,
                   num_devices=NCORES)
    x_d = nc.dram_tensor("xk", [4, KH, 128, BH], BF16, kind="ExternalInput")
    w_d = nc.dram_tensor("w", [7, JH, 128, KH, 128], BF16,
                         kind="ExternalInput")
    b_d = nc.dram_tensor("bias", [128, JT], F32, kind="ExternalInput")
    o_d = nc.dram_tensor("out", [JT, 128, B], BF16,
                         kind="ExternalOutput")

    with tile.TileContext(nc) as tc:
        with (
            tc.tile_pool(name="xpool", bufs=1) as xpool,
            tc.tile_pool(name="xtr", bufs=4) as xtr,
            tc.tile_pool(name="spool", bufs=1) as spool,
            tc.tile_pool(name="wpool", bufs=1) as wpool,
            tc.tile_pool(name="bpool", bufs=1) as bpool,
            tc.tile_pool(name="opool", bufs=2) as opool,
            tc.tile_pool(name="pspool", bufs=1, space="PSUM") as pspool,
        ):
            bias_t = bpool.tile([128, JT], F32, tag="bias", name="bias_t")
            nc.scalar.dma_start(out=bias_t[:], in_=b_d[:])

            # x^T blocks: A11, A22 stay resident; A21, A12 pass through
            # rotating tiles feeding the DVE sum builders.
            a11, a22 = [], []
            s1, s2, s5, s6, s7 = [], [], [], [], []
            for k in range(KH):
                t11 = xpool.tile([128, BH], BF16, tag=f"a11_{k}",
                                 name=f"a11_{k}")
                t22 = xpool.tile([128, BH], BF16, tag=f"a22_{k}",
                                 name=f"a22_{k}")
                nc.sync.dma_start(out=t11[:], in_=x_d[0, k])
                nc.gpsimd.dma_start(out=t22[:], in_=x_d[1, k])
                a11.append(t11)
                a22.append(t22)
                st = spool.tile([128, BH], BF16, tag=f"s1_{k}",
                                name=f"s1_{k}")
                nc.vector.tensor_tensor(st[:], t11[:], t22[:], ADD)
                s1.append(st)
            for k in range(KH):
                t21 = xtr.tile([128, BH], BF16, tag="a21", name=f"a21_{k}")
                nc.gpsimd.dma_start(out=t21[:], in_=x_d[2, k])
                st2 = spool.tile([128, BH], BF16, tag=f"s2_{k}",
                                 name=f"s2_{k}")
                st6 = spool.tile([128, BH], BF16, tag=f"s6_{k}",
                                 name=f"s6_{k}")
                nc.vector.tensor_tensor(st2[:], t21[:], a22[k][:], ADD)
                nc.vector.tensor_tensor(st6[:], t21[:], a11[k][:], SUB)
                s2.append(st2)
                s6.append(st6)
            for k in range(KH):
                t12 = xtr.tile([128, BH], BF16, tag="a12", name=f"a12_{k}")
                nc.sync.dma_start(out=t12[:], in_=x_d[3, k])
                st5 = spool.tile([128, BH], BF16, tag=f"s5_{k}",
                                 name=f"s5_{k}")
                st7 = spool.tile([128, BH], BF16, tag=f"s7_{k}",
                                 name=f"s7_{k}")
                nc.vector.tensor_tensor(st5[:], a11[k][:], t12[:], ADD)
                nc.vector.tensor_tensor(st7[:], t12[:], a22[k][:], SUB)
                s5.append(st5)
                s7.append(st7)

            moving = {"s1": s1, "s2": s2, "a11": a11, "a22": a22,
                      "s5": s5, "s6": s6, "s7": s7}

            # chains run in pairs: consecutive matmuls alternate between
            # two chains (separate full PSUM banks), so each LDWEIGHTS
            # hides under the other chain's 512-col matmul.
            PAIRS = [(0, 1), (2, 3), (4, 5), (6,)]
            for j in range(JH):
                # stream the 7 stationary slabs for this o-tile, each in
                # 4 chunks so the first chain gates on 128 KB not 512 KB
                wts = {}
                for i, mv, _ in CHAINS:
                    wt = wpool.tile([128, KH, 128], BF16, tag=f"w{i}",
                                    name=f"w{i}")
                    for q in range(4):
                        nc.scalar.dma_start(
                            out=wt[:, 4 * q:4 * (q + 1), :],
                            in_=w_d[i, j, :, 4 * q:4 * (q + 1)])
                    wts[i] = wt

                acc = {}
                done = {d: 0 for d in NTERMS}
                for pair in PAIRS:
                    members = [CHAINS[c] for c in pair]
                    pss = {}
                    for i, mv, _ in members:
                        pss[i] = pspool.tile([128, BH], F32, tag=f"m{i}",
                                             name=f"m{i}")
                    for k in range(KH):
                        for i, mv, _ in members:
                            nc.tensor.matmul(
                                pss[i][:], wts[i][:, k, :],
                                moving[mv][k][:],
                                start=(k == 0), stop=(k == KH - 1),
                            )
                    # fold each M_i into its SBUF accumulators (frees
                    # the PSUM bank once the reads retire); the final
                    # fold of each accumulator writes bf16 to halve the
                    # output DMA traffic
                    for i, mv, dsts in members:
                        ps = pss[i]
                        for dst, sign in dsts:
                            ohalf, bhalf = DSTS[dst]
                            final = done[dst] + 1 == NTERMS[dst]
                            if dst not in acc:
                                acc[dst] = opool.tile([128, BH], F32,
                                                      tag=dst, name=dst)
                                assert sign > 0 and not final
                                nc.vector.tensor_scalar_add(
                                    acc[dst][:], ps[:],
                                    bias_t[:, 16 * ohalf + j:
                                           16 * ohalf + j + 1],
                                )
                            elif final:
                                ob = opool.tile([128, BH], BF16,
                                                tag=dst + "b",
                                                name=dst + "b")
                                nc.vector.tensor_tensor(
                                    ob[:], acc[dst][:], ps[:],
                                    ADD if sign > 0 else SUB,
                                )
                                nc.sync.dma_start(
                                    out=o_d[16 * ohalf + j, :,
                                            BH * bhalf:BH * (bhalf + 1)],
                                    in_=ob[:],
                                )
                            else:
                                nc.vector.tensor_tensor(
                                    acc[dst][:], acc[dst][:], ps[:],
                                    ADD if sign > 0 else SUB,
                                )
                            done[dst] += 1

    nc.compile()
    _NC_CACHE["nc"] = nc
    return nc


def kernel(x, weight, bias):
    x = np.asarray(x, dtype=np.float32)
    weight = np.asarray(weight, dtype=np.float32)
    bias = np.asarray(bias, dtype=np.float32)

    nc = _build_nc()

    # host-side Strassen weight sums (fp32 exact, single bf16 rounding)
    WT = weight.T  # [K, O]
    K1, K2 = slice(0, 2048), slice(2048, 4096)
    O1, O2 = slice(0, 2048), slice(2048, 4096)
    B11, B12 = WT[K1, O1], WT[K1, O2]
    B21, B22 = WT[K2, O1], WT[K2, O2]
    Ts = [B11 + B22, B11, B12 - B22, B21 - B11, B22, B11 + B12, B21 + B22]
    # [7, 2048 K, 2048 O] -> [7, j, k_local, k_tile, o_local]
    wr = np.empty((7, JH, 128, KH, 128), dtype=ml_dtypes.bfloat16)
    for i, T in enumerate(Ts):
        wr[i] = (T.astype(ml_dtypes.bfloat16)
                 .reshape(KH, 128, JH, 128).transpose(2, 1, 0, 3))
    br = np.ascontiguousarray(bias.reshape(JT, 128).T)

    in_maps = []
    for c in range(NCORES):
        xs = x[c * B:(c + 1) * B].astype(ml_dtypes.bfloat16)
        xb = np.empty((4, KH, 128, BH), dtype=ml_dtypes.bfloat16)
        for bi, (rs, cs) in enumerate(
                [(slice(0, BH), K1), (slice(BH, B), K2),
                 (slice(BH, B), K1), (slice(0, BH), K2)]):
            # order: A11, A22, A21, A12
            xb[bi] = xs[rs, cs].T.reshape(KH, 128, BH)
        in_maps.append({"xk": xb, "w": wr, "bias": br})

    res = run_bass_kernel_spmd(nc, in_maps, list(range(NCORES)))

    out = np.empty((BATCH, OUTF), np.float32)
    for c in range(NCORES):
        out[c * B:(c + 1) * B] = (res.results[c]["out"]
                                  .astype(np.float32).reshape(OUTF, B).T)
    return out


# revision 19
# speedup vs baseline: 1.1834x; 1.0104x over previous
"""BlockSparseLinear on 8 TRN2 NeuronCores: out = x @ W^T + bias.

Harness entry point: kernel(**inputs) takes the FULL inputs
(x (8192,4096) f32, weight (4096,4096) f32, bias (4096,) f32) and
returns the FULL output (8192,4096) f32.

Strategy: 8-way data parallel over batch + one level of Strassen, all
in bf16. Per core the shard out^T[:, b] = W @ x^T[:, b] + bias is a
[1024 x 4096] @ [4096 x 4096] product; Strassen splits batch (a/b
halves of 512), contraction K (halves of 2048) and outputs O (halves
of 2048) in 2x2 blocks and runs 7 block-products instead of 8 -> PE
streaming work drops 12.5% below the dense bf16 floor.

  A11 = x_a[:,K1]  A12 = x_a[:,K2]  A21 = x_b[:,K1]  A22 = x_b[:,K2]
  M1=(A11+A22)(B11+B22) M2=(A21+A22)B11 M3=A11(B12-B22)
  M4=A22(B21-B11)       M5=(A11+A12)B22 M6=(A21-A11)(B11+B12)
  M7=(A12-A22)(B21+B22)
  C11=M1+M4-M5+M7  C12=M3+M5  C21=M2+M4  C22=M1-M2+M3+M6

The 7 weight-sum matrices T_i are precomputed on the host in fp32 and
rounded once to bf16 (56 MB streamed per core). The 5 x-sums are built
on the DVE from the x^T shard as it lands. Each of the 7 M-products
accumulates 16 k-tiles into its own full PSUM bank; after a chain
completes, the DVE folds that M into the SBUF accumulators c11..c22
(bias folded into the first read, one PSUM operand per op - PSUM has a
single DVE read port), freeing the bank for the next output block
while the PE runs the remaining chains. Measured numerically on the
real data the scheme lands at ~5e-3 rel err vs the 2e-2 gate.

Layouts (host pre-blocked so every DMA is contiguous per partition):
  xk   (4, 16, 128, 512)        x^T blocks [A11,A22,A21,A12] as
                                [blk, k_tile, k_local, b_half]   bf16
  w    (7, 16, 128, 16, 128)    T_i blocked [i, j_tile, k_local,
                                k_tile, o_local]                 bf16
  bias (128, 32)                [o_local, j_tile]                f32
  out  (32, 128, 1024)          out^T blocked [j_tile, o_local, b] bf16
"""

import ml_dtypes
import numpy as np

import concourse.mybir as mybir
import concourse.tile as tile
from concourse import bacc
from concourse.bass_utils import run_bass_kernel_spmd

NCORES = 8
BATCH, INF, OUTF = 8192, 4096, 4096
B = BATCH // NCORES          # per-core batch (1024)
BH = B // 2                  # Strassen batch half (512)
KH = 16                      # k-tiles per K-half (of 128 features)
JH = 16                      # o-tiles per O-half (of 128 outputs)
JT = OUTF // 128             # 32 o-tiles total

F32 = mybir.dt.float32
BF16 = mybir.dt.bfloat16
ADD = mybir.AluOpType.add
SUB = mybir.AluOpType.subtract

_NC_CACHE = {}

# chain order i -> (moving operand, which accumulators consume M_i and
# with what sign). Moving operands: S1,S2,A11,A22,S5,S6,S7.
# c11 = M1+M4-M5+M7   c12 = M3+M5   c21 = M2+M4   c22 = M1-M2+M3+M6
CHAINS = [
    # (w index i, moving key, [(dst, sign), ...])
    (0, "s1", [("c11", +1), ("c22", +1)]),   # M1
    (2, "a11", [("c12", +1), ("c22", +1)]),  # M3
    (3, "a22", [("c11", +1), ("c21", +1)]),  # M4
    (1, "s2", [("c21", +1), ("c22", -1)]),   # M2
    (5, "s6", [("c22", +1)]),                # M6
    (4, "s5", [("c12", +1), ("c11", -1)]),   # M5
    (6, "s7", [("c11", +1)]),                # M7
]
# which bias column (j for O1, 16+j for O2) and out rows per dst
DSTS = {
    "c11": (0, 0),    # (o-half, batch-half)
    "c12": (1, 0),
    "c21": (0, 1),
    "c22": (1, 1),
}
# number of M-terms per dst (to know when it is complete)
NTERMS = {"c11": 4, "c12": 2, "c21": 2, "c22": 4}


def _build_nc():
    if "nc" in _NC_CACHE:
        return _NC_CACHE["nc"]
    nc = bacc.Bacc("TRN2", target_bir_lowering=False, debug=False,
                   num_devices=NCORES)
    x_d = nc.dram_tensor("xk", [4, KH // 2, 128, 2, BH], BF16,
                         kind="ExternalInput")
    w_d = nc.dram_tensor("w", [7, JH, 128, KH, 128], BF16,
                         kind="ExternalInput")
    b_d = nc.dram_tensor("bias", [128, JT], F32, kind="ExternalInput")
    o_d = nc.dram_tensor("out", [JT, 128, B], BF16,
                         kind="ExternalOutput")

    with tile.TileContext(nc) as tc:
        with (
            tc.tile_pool(name="xpool", bufs=1) as xpool,
            tc.tile_pool(name="xtr", bufs=4) as xtr,
            tc.tile_pool(name="spool", bufs=1) as spool,
            tc.tile_pool(name="wpool", bufs=1) as wpool,
            tc.tile_pool(name="bpool", bufs=1) as bpool,
            tc.tile_pool(name="opool", bufs=2) as opool,
            tc.tile_pool(name="pspool", bufs=1, space="PSUM") as pspool,
        ):
            bias_t = bpool.tile([128, JT], F32, tag="bias", name="bias_t")
            nc.scalar.dma_start(out=bias_t[:], in_=b_d[:])

            # x^T blocks: A11, A22 stay resident; A21, A12 pass through
            # rotating tiles feeding the DVE sum builders.
            a11, a22 = [], []
            s1, s2, s5, s6, s7 = [], [], [], [], []
            for k in range(KH // 2):
                t11 = xpool.tile([128, 2, BH], BF16, tag=f"a11_{k}",
                                 name=f"a11_{k}")
                t22 = xpool.tile([128, 2, BH], BF16, tag=f"a22_{k}",
                                 name=f"a22_{k}")
                nc.sync.dma_start(out=t11[:], in_=x_d[0, k])
                nc.gpsimd.dma_start(out=t22[:], in_=x_d[1, k])
                a11.append(t11)
                a22.append(t22)
                st = spool.tile([128, 2, BH], BF16, tag=f"s1_{k}",
                                name=f"s1_{k}")
                nc.vector.tensor_tensor(st[:], t11[:], t22[:], ADD)
                s1.append(st)
            for k in range(KH // 2):
                t21 = xtr.tile([128, 2, BH], BF16, tag="a21",
                               name=f"a21_{k}")
                nc.gpsimd.dma_start(out=t21[:], in_=x_d[2, k])
                st2 = spool.tile([128, 2, BH], BF16, tag=f"s2_{k}",
                                 name=f"s2_{k}")
                st6 = spool.tile([128, 2, BH], BF16, tag=f"s6_{k}",
                                 name=f"s6_{k}")
                nc.vector.tensor_tensor(st2[:], t21[:], a22[k][:], ADD)
                nc.vector.tensor_tensor(st6[:], t21[:], a11[k][:], SUB)
                s2.append(st2)
                s6.append(st6)
            for k in range(KH // 2):
                t12 = xtr.tile([128, 2, BH], BF16, tag="a12",
                               name=f"a12_{k}")
                nc.sync.dma_start(out=t12[:], in_=x_d[3, k])
                st5 = spool.tile([128, 2, BH], BF16, tag=f"s5_{k}",
                                 name=f"s5_{k}")
                st7 = spool.tile([128, 2, BH], BF16, tag=f"s7_{k}",
                                 name=f"s7_{k}")
                nc.vector.tensor_tensor(st5[:], a11[k][:], t12[:], ADD)
                nc.vector.tensor_tensor(st7[:], t12[:], a22[k][:], SUB)
                s5.append(st5)
                s7.append(st7)

            moving = {"s1": s1, "s2": s2, "a11": a11, "a22": a22,
                      "s5": s5, "s6": s6, "s7": s7}

            # chains run in pairs: consecutive matmuls alternate between
            # two chains (separate full PSUM banks), so each LDWEIGHTS
            # hides under the other chain's 512-col matmul.
            PAIRS = [(0, 1), (2, 3), (4, 5), (6,)]
            for j in range(JH):
                # stream the 7 stationary slabs for this o-tile, each in
                # 4 chunks so the first chain gates on 128 KB not 512 KB
                wts = {}
                for i, mv, _ in CHAINS:
                    wt = wpool.tile([128, KH, 128], BF16, tag=f"w{i}",
                                    name=f"w{i}")
                    for q in range(2):
                        nc.scalar.dma_start(
                            out=wt[:, 8 * q:8 * (q + 1), :],
                            in_=w_d[i, j, :, 8 * q:8 * (q + 1)])
                    wts[i] = wt

                acc = {}
                done = {d: 0 for d in NTERMS}
                for pair in PAIRS:
                    members = [CHAINS[c] for c in pair]
                    pss = {}
                    for i, mv, _ in members:
                        pss[i] = pspool.tile([128, BH], F32, tag=f"m{i}",
                                             name=f"m{i}")
                    for k in range(KH):
                        for i, mv, _ in members:
                            nc.tensor.matmul(
                                pss[i][:], wts[i][:, k, :],
                                moving[mv][k // 2][:, k % 2, :],
                                start=(k == 0), stop=(k == KH - 1),
                            )
                    # fold each M_i into its SBUF accumulators (frees
                    # the PSUM bank once the reads retire); the final
                    # fold of each accumulator writes bf16 to halve the
                    # output DMA traffic
                    for i, mv, dsts in members:
                        ps = pss[i]
                        for dst, sign in dsts:
                            ohalf, bhalf = DSTS[dst]
                            final = done[dst] + 1 == NTERMS[dst]
                            if dst not in acc:
                                acc[dst] = opool.tile([128, BH], F32,
                                                      tag=dst, name=dst)
                                assert sign > 0 and not final
                                nc.vector.tensor_scalar_add(
                                    acc[dst][:], ps[:],
                                    bias_t[:, 16 * ohalf + j:
                                           16 * ohalf + j + 1],
                                )
                            elif final:
                                ob = opool.tile([128, BH], BF16,
                                                tag=dst + "b",
                                                name=dst + "b")
                                nc.vector.tensor_tensor(
                                    ob[:], acc[dst][:], ps[:],
                                    ADD if sign > 0 else SUB,
                                )
                                nc.sync.dma_start(
                                    out=o_d[16 * ohalf + j, :,
                                            BH * bhalf:BH * (bhalf + 1)],
                                    in_=ob[:],
                                )
                            else:
                                nc.vector.tensor_tensor(
                                    acc[dst][:], acc[dst][:], ps[:],
                                    ADD if sign > 0 else SUB,
                                )
                            done[dst] += 1

    nc.compile()
    _NC_CACHE["nc"] = nc
    return nc


def kernel(x, weight, bias):
    x = np.asarray(x, dtype=np.float32)
    weight = np.asarray(weight, dtype=np.float32)
    bias = np.asarray(bias, dtype=np.float32)

    nc = _build_nc()

    # host-side Strassen weight sums (fp32 exact, single bf16 rounding)
    WT = weight.T  # [K, O]
    K1, K2 = slice(0, 2048), slice(2048, 4096)
    O1, O2 = slice(0, 2048), slice(2048, 4096)
    B11, B12 = WT[K1, O1], WT[K1, O2]
    B21, B22 = WT[K2, O1], WT[K2, O2]
    Ts = [B11 + B22, B11, B12 - B22, B21 - B11, B22, B11 + B12, B21 + B22]
    # [7, 2048 K, 2048 O] -> [7, j, k_local, k_tile, o_local]
    wr = np.empty((7, JH, 128, KH, 128), dtype=ml_dtypes.bfloat16)
    for i, T in enumerate(Ts):
        wr[i] = (T.astype(ml_dtypes.bfloat16)
                 .reshape(KH, 128, JH, 128).transpose(2, 1, 0, 3))
    br = np.ascontiguousarray(bias.reshape(JT, 128).T)

    in_maps = []
    for c in range(NCORES):
        xs = x[c * B:(c + 1) * B].astype(ml_dtypes.bfloat16)
        xb = np.empty((4, KH // 2, 128, 2, BH), dtype=ml_dtypes.bfloat16)
        for bi, (rs, cs) in enumerate(
                [(slice(0, BH), K1), (slice(BH, B), K2),
                 (slice(BH, B), K1), (slice(0, BH), K2)]):
            # order: A11, A22, A21, A12; k-tile pairs per partition
            # line so each DMA line is 2 KB
            xb[bi] = (xs[rs, cs].T.reshape(KH // 2, 2, 128, BH)
                      .transpose(0, 2, 1, 3))
        in_maps.append({"xk": xb, "w": wr, "bias": br})

    res = run_bass_kernel_spmd(nc, in_maps, list(range(NCORES)))

    out = np.empty((BATCH, OUTF), np.float32)
    for c in range(NCORES):
        out[c * B:(c + 1) * B] = (res.results[c]["out"]
                                  .astype(np.float32).reshape(OUTF, B).T)
    return out
